# revision 1
# baseline (speedup 1.0000x reference)
"""Bass/Trainium2 kernel for nn_Block_14010183320003 (MST++-style block).

Sharding: 8 cores = 2 batches x 4 row-quarters (64 owned rows each, halo 6).
Chain packing: partitions 0:62 = x-derived chain A, 64:126 = y-derived chain B.
Attention is computed from 124-dim input Gram matrices (no q/k tensors);
per-core partial Grams are summed with two ~256KB AllReduces.
Row layout: stride 258 per row (2 zero pad cols) so 3x3 convs are free-axis
shifted reads; depthwise convs run as PE diag-matmul accumulation or DVE
scalar_tensor_tensor chains per engine flags.
"""
import sys

sys.path.insert(0, "/opt/trn_rl_repo")
import numpy as np
import ml_dtypes

import concourse.bass as bass
import concourse.mybir as mybir
import concourse.tile as tile
import concourse.bacc as bacc
from concourse.bass_utils import run_bass_kernel_spmd

BF16, F32 = mybir.dt.bfloat16, mybir.dt.float32
AF = mybir.ActivationFunctionType
ALU = mybir.AluOpType
bf = ml_dtypes.bfloat16

C = 62
W = 256
RS = 258  # row stride (2 zero pad cols)
OWN = 64
HALO = 6
R = 76
ROFF = 1  # lead pad row at tile row 0 (zero pad for shifted reads)
RA = 78  # lead pad + 76 slab rows + 1 trailing spare
FA = RA * RS  # 20124
NT = 512  # matmul moving chunk
EV = 2048  # evacuation chunk
NH, DH = 2, 31

# conv engine flags per site: "pe" or "dve"
ENG = {
    "dw1_01": "pe", "dw2_01": "pe",
    "dw1_23": "pe", "dw2_23": "pe",
    "dw1_45": "pe", "dw2_45": "pe",
    "ff_a0": "pe", "ff_a1": "pe", "ff_b0": "pe", "ff_b1": "pe",
}

TAPS = [(dy, dx) for dy in (-1, 0, 1) for dx in (-1, 0, 1)]


def exr(e):
    """free range of extent e (slab rows [6-e, 70+e)), incl lead-row offset."""
    return ((HALO - e + ROFF) * RS, (HALO + OWN + e + ROFF) * RS)


def _slab(full, row0):
    """full [C, 256, 256] -> [C, R, RS] zero-padded slab (rows row0..row0+R)."""
    out = np.zeros((C, R, RS), np.float32)
    lo, hi = max(0, row0), min(256, row0 + R)
    out[:, lo - row0 : hi - row0, :W] = full[:, lo:hi]
    return out


def _to_bf(a):
    return np.ascontiguousarray(a.astype(bf))


def _diag_taps(kvec_by_tap):
    """kvec_by_tap: [9, 128] -> [128, 9*128] bf16 block of diag matrices."""
    out = np.zeros((128, 9 * 128), np.float32)
    for t in range(9):
        out[:, t * 128 : (t + 1) * 128][np.arange(128), np.arange(128)] = kvec_by_tap[t]
    return _to_bf(out)


def _pair128(a_block, b_block):
    """[62, m] + [62, m] -> [128, 2m-ish] block-diag lhsT [128, 128] style:
    cols 0:62 <- a at rows 0:62; cols 64:126 <- b at rows 64:126."""
    out = np.zeros((128, 128), np.float32)
    out[0:62, 0:62] = a_block
    out[64:126, 64:126] = b_block
    return _to_bf(out)


def prep_host_inputs(inputs):
    """Build the 8 per-core input maps."""
    inp = {k: np.asarray(v, dtype=np.float32) for k, v in inputs.items()}
    wq, wk, wv = inp["attn_wq"], inp["attn_wk"], inp["attn_wv"]
    pw, pb = inp["attn_pw"], inp["attn_pb"]
    dw1, dw2 = inp["attn_dw1"], inp["attn_dw2"]
    resc = inp["attn_rescale"]

    shared = {}
    # v-projection pair weights [128,128]: cols 0:62 = wv_even (K rows 0:62),
    # cols 64:126 = wv_odd (K rows 64:126). lhsT[k, m] = wv[k, m].
    for pair, (me, mo) in {"01": (0, 1), "23": (2, 3), "45": (4, 5)}.items():
        shared[f"wv{pair}"] = _pair128(wv[me], wv[mo])
        pbv = np.zeros((128, 1), np.float32)
        pbv[0:62, 0] = pb[me]
        pbv[64:126, 0] = pb[mo]
        shared[f"pb{pair}"] = pbv
        for cn, dwk in (("1", dw1), ("2", dw2)):
            kc = np.zeros((128, 9), np.float32)
            for t, (dy, dx) in enumerate(TAPS):
                kc[0:62, t] = dwk[me][:, dy + 1, dx + 1]
                kc[64:126, t] = dwk[mo][:, dy + 1, dx + 1]
            shared[f"c_dw{cn}_{pair}"] = kc
            shared[f"m_dw{cn}_{pair}"] = _diag_taps(kc.T)
    # attn small weights: [128, 6*64] col block m at m*64, rows 0:62
    for nm, src in (("wqm", wq), ("wkm", wk), ("pwm", pw)):
        t = np.zeros((128, 6 * 64), np.float32)
        for m in range(6):
            t[0:62, m * 64 : m * 64 + 62] = src[m]
        shared[nm] = t
    rs = np.zeros((128, 6), np.float32)
    for m in range(6):
        rs[0:62, m] = np.repeat(resc[m], DH)
    shared["rsc"] = rs
    ones62 = np.zeros((128, 1), np.float32)
    ones62[0:62, 0] = 1.0
    shared["ones62"] = ones62
    hmA = np.zeros((128, 64), np.float32)
    hmB = np.full((128, 64), -1e4, np.float32)
    for h in range(NH):
        hmA[h * DH : (h + 1) * DH, h * DH : (h + 1) * DH] = 1.0
        hmB[h * DH : (h + 1) * DH, h * DH : (h + 1) * DH] = 0.0
    shared["hmA"] = hmA
    shared["hmB"] = hmB
    lnones = np.zeros((128, 2), np.float32)
    lnones[0:62, 0] = 1.0 / C
    lnones[64:126, 1] = 1.0 / C
    shared["lnones"] = _to_bf(lnones)

    # FF weights. chain A uses ff index 0, chain B index 1.
    # w1aug[(c)(h)]: [128, 124]: rows (chain rows) = diag(g)@w1 half,
    # row 62/126 = -colsum (mean-subtraction augmentation).
    for ci, cname in ((0, "a"), (1, "b")):
        g, b = inp["ln_g"][ci], inp["ln_b"][ci]
        w1 = inp["ff_w1"][ci]  # [62, 248]
        w2 = inp["ff_w2"][ci]  # [248, 62]
        w1p = g[:, None] * w1
        c2 = w1p.sum(axis=0)  # [248]
        c1 = b @ w1  # [248]
        dwk = inp["ff_dw"][ci]  # [248, 3, 3]
        r0 = 0 if ci == 0 else 64
        for h in (0, 1):
            sl = slice(h * 124, (h + 1) * 124)
            wt = np.zeros((128, 124), np.float32)
            wt[r0 : r0 + 62, :] = w1p[:, sl]
            wt[r0 + 62, :] = -c2[sl]
            shared[f"w1aug_{cname}{h}"] = _to_bf(wt)
            cb = np.zeros((128, 1), np.float32)
            cb[0:124, 0] = c1[sl]
            shared[f"c1b_{cname}{h}"] = cb
            kc = np.zeros((128, 9), np.float32)
            for t, (dy, dx) in enumerate(TAPS):
                kc[0:124, t] = dwk[sl, dy + 1, dx + 1]
            shared[f"c_ffdw_{cname}{h}"] = kc
            shared[f"m_ffdw_{cname}{h}"] = _diag_taps(kc.T)
            w2t = np.zeros((128, 128), np.float32)
            w2t[0:124, 0:62] = w2[sl, :]
            shared[f"w2h_{cname}{h}"] = _to_bf(w2t)

    # fusion weights
    fu1 = np.zeros((128, 128), np.float32)
    fu1[0:62, 0:62] = inp["fuc1_w"][:, 0:62].T  # t1f channels
    fu1[64:126, 0:62] = inp["fuc1_w"][:, 62:124].T  # xa channels
    shared["fu1w"] = _to_bf(fu1)
    fu2 = np.zeros((128, 128), np.float32)
    fu2[0:62, 64:126] = inp["fuc2_w"][:, 0:62].T
    fu2[64:126, 64:126] = inp["fuc2_w"][:, 62:124].T
    shared["fu2w"] = _to_bf(fu2)
    fucb = np.zeros((128, 1), np.float32)
    fucb[0:62, 0] = inp["fuc1_b"]
    fucb[64:126, 0] = inp["fuc2_b"]
    shared["fucb"] = fucb
    ow = np.zeros((128, 64), np.float32)
    ow[0:62, 0:62] = inp["outc_w"][:, 0:62].T
    ow[64:126, 0:62] = inp["outc_w"][:, 62:124].T
    shared["outw"] = _to_bf(ow)
    ob = np.zeros((128, 1), np.float32)
    ob[0:62, 0] = inp["outc_b"]
    shared["outb"] = ob
    fw = np.zeros((128, 9 * 64), np.float32)
    for t, (dy, dx) in enumerate(TAPS):
        fw[0:62, t * 64 : t * 64 + 62] = inp["final_w"][:, 0:62, dy + 1, dx + 1].T
        fw[64:126, t * 64 : t * 64 + 62] = inp["final_w"][:, 62:124, dy + 1, dx + 1].T
    shared["finw"] = _to_bf(fw)
    fb_ = np.zeros((128, 1), np.float32)
    fb_[0:62, 0] = inp["final_b"]
    shared["finb"] = fb_

    float_keys = {k for k, v in shared.items() if v.dtype == np.float32}
    for k in list(shared.keys()):
        if shared[k].dtype == np.float32:
            shared[k] = np.ascontiguousarray(shared[k])

    in_maps = []
    for b in range(2):
        xb, yb_, zb = inp["x"][b], inp["y"][b], inp["z"][b]
        for s in range(4):
            row0 = 64 * s - HALO
            xs, ys, zs = _slab(xb, row0), _slab(yb_, row0), _slab(zb, row0)
            m = {}
            xy = np.zeros((128, R, RS), np.float32)
            xy[0:62], xy[64:126] = xs, ys
            m["xy"] = _to_bf(xy.reshape(128, R * RS))
            zz = np.zeros((128, R, RS), np.float32)
            zz[0:62], zz[64:126] = zs, zs
            m["zz"] = _to_bf(zz.reshape(128, R * RS))
            m["zc"] = _to_bf(
                np.concatenate([zs[:, 5:71].reshape(62, 66 * RS),
                                np.zeros((66, 66 * RS), np.float32)], 0)
            )
            # pixel-major Gram stacks over owned real pixels [16384, 128]
            def pixmaj(t1, t2):
                g = np.zeros((OWN * W, 128), np.float32)
                g[:, 0:62] = t1[:, HALO : HALO + OWN, :W].reshape(C, -1).T
                g[:, 64:126] = t2[:, HALO : HALO + OWN, :W].reshape(C, -1).T
                return _to_bf(g)
            m["gzx"] = pixmaj(zs, xs)
            m["gzy"] = pixmaj(zs, ys)
            msk = np.zeros((128, 4), np.float32)
            msk[:, 0] = 0.0 if s == 0 else 1.0
            msk[:, 1] = 0.0 if s == 3 else 1.0
            msk[:, 2] = 1.0 if b == 0 else 0.0
            msk[:, 3] = 1.0 if b == 1 else 0.0
            m["msk"] = msk
            m.update(shared)
            in_maps.append(m)
    return in_maps


def assemble_output(results):
    out = np.zeros((2, C, 256, 256), np.float32)
    for b in range(2):
        for s in range(4):
            r = results[b * 4 + s]["out"]  # [62, 64*256]
            out[b, :, 64 * s : 64 * (s + 1), :] = r.reshape(C, OWN, W)
    return out


# ---------------------------------------------------------------------------
# device IR
# ---------------------------------------------------------------------------

PAIRS = ["01", "23", "45"]
L2LEN = exr(2)[1] - exr(2)[0]  # 17544


def build_nc():
    nc = bacc.Bacc(None, target_bir_lowering=False, debug=False)

    P = {}
    for nm in ["xy", "zz"]:
        P[nm] = nc.declare_dram_parameter(nm, [128, R * RS], BF16, isOutput=False)
    P["zc"] = nc.declare_dram_parameter("zc", [128, 66 * RS], BF16, isOutput=False)
    P["gzx"] = nc.declare_dram_parameter("gzx", [OWN * W, 128], BF16, isOutput=False)
    P["gzy"] = nc.declare_dram_parameter("gzy", [OWN * W, 128], BF16, isOutput=False)
    P["msk"] = nc.declare_dram_parameter("msk", [128, 4], F32, isOutput=False)
    wnames_bf = (
        [f"wv{p}" for p in PAIRS]

        + ["lnones"]
        + [f"w1aug_{c}{h}" for c in "ab" for h in "01"]
        + [f"w2h_{c}{h}" for c in "ab" for h in "01"]
        + ["fu1w", "fu2w", "outw", "finw"]
    )
    wnames_f32 = (
        [f"pb{p}" for p in PAIRS]
        + [f"c_dw{c}_{p}" for c in "12" for p in PAIRS]
        + ["rsc", "wqm", "wkm", "pwm", "ones62", "hmA", "hmB"]
        + [f"c_ffdw_{c}{h}" for c in "ab" for h in "01"]
        + [f"c1b_{c}{h}" for c in "ab" for h in "01"]
        + ["fucb", "outb", "finb"]
    )
    WSHAPE = {
        "wqm": [128, 6 * 64], "wkm": [128, 6 * 64], "pwm": [128, 6 * 64],
        "ones62": [128, 1], "lnones": [128, 2], "rsc": [128, 6],
        "outw": [128, 64], "finw": [128, 9 * 64],
        "hmA": [128, 64], "hmB": [128, 64],
        "fu1w": [128, 128], "fu2w": [128, 128],
        "fucb": [128, 1], "outb": [128, 1], "finb": [128, 1],
    }
    for p in PAIRS:
        WSHAPE[f"wv{p}"] = [128, 128]
        WSHAPE[f"pb{p}"] = [128, 1]
        for c in "12":
            WSHAPE[f"m_dw{c}_{p}"] = [128, 9 * 128]
            WSHAPE[f"c_dw{c}_{p}"] = [128, 9]
    for c in "ab":
        for h in "01":
            WSHAPE[f"w1aug_{c}{h}"] = [128, 124]
            WSHAPE[f"m_ffdw_{c}{h}"] = [128, 9 * 128]
            WSHAPE[f"c_ffdw_{c}{h}"] = [128, 9]
            WSHAPE[f"c1b_{c}{h}"] = [128, 1]
            WSHAPE[f"w2h_{c}{h}"] = [128, 128]
    mnames = [f"m_dw{c}_{p}" for c in "12" for p in PAIRS
              if ENG[f"dw{c}_{p}"] == "pe"] + [
        f"m_ffdw_{c}{h}" for c in "ab" for h in "01" if ENG[f"ff_{c}{h}"] == "pe"
    ]
    for nm in wnames_bf + mnames:
        P[nm] = nc.declare_dram_parameter(nm, WSHAPE[nm], BF16, isOutput=False)
    for nm in wnames_f32:
        P[nm] = nc.declare_dram_parameter(nm, WSHAPE[nm], F32, isOutput=False)
    out_p = nc.declare_dram_parameter("out", [C, OWN * W], F32, isOutput=True)

    with tile.TileContext(nc, num_cores=8) as tc:
        with (
            tc.tile_pool(name="w", bufs=1) as wp,
            tc.tile_pool(name="small", bufs=1) as sp,
            tc.tile_pool(name="big", bufs=1) as bp,
            tc.tile_pool(name="ring", bufs=3) as rp,
            tc.tile_pool(name="psA", bufs=2, space="PSUM") as psA,
            tc.tile_pool(name="psB", bufs=1, space="PSUM") as psB,
            tc.tile_pool(name="psC", bufs=1, space="PSUM") as psC,
            tc.tile_pool(name="psD", bufs=1, space="PSUM") as psD,
            tc.tile_pool(name="dram", bufs=1, space="DRAM") as dp,
        ):
            WT = {}
            for nm in wnames_bf:
                WT[nm] = wp.tile(WSHAPE[nm], BF16, tag=nm, name=nm)
                nc.sync.dma_start(WT[nm][:], P[nm][:])
            for nm in wnames_f32:
                WT[nm] = wp.tile(WSHAPE[nm], F32, tag=nm, name=nm)
                nc.sync.dma_start(WT[nm][:], P[nm][:])
            msk = sp.tile([128, 4], F32, tag="msk")
            nc.sync.dma_start(msk[:], P["msk"][:])
            SEL0, SEL1 = msk[:, 2:3], msk[:, 3:4]
            MTOP, MBOT = msk[:, 0:1], msk[:, 1:2]

            def load_mdw(nm):
                t_ = rp.tile([128, 9 * 128], BF16, tag="mdw", bufs=2, name=nm + "_l")
                nc.sync.dma_start(t_[:], P[nm][:])
                return t_

            def chunks(rng, step):
                lo, hi = rng
                out = []
                while lo < hi:
                    out.append((lo, min(lo + step, hi)))
                    lo += step
                return out

            def pad_zero(t, lo_row=0, hi_row=RA):
                """zero pad cols 256,257 of rows [lo_row, hi_row) of tile t
                (tile assumed row-aligned at slab row 0)."""
                v = t[:].rearrange("p (r s) -> p r s", s=RS)[:, lo_row:hi_row, W : W + 2]
                nc.vector.memset(v, 0.0)

            def mask_rows(t, e, dtype_rows=(0, 128)):
                """zero out-of-image rows within extent e via per-core scalars."""
                if e <= 0:
                    return
                r0, r1 = dtype_rows
                top = t[r0:r1, (HALO - e + ROFF) * RS : (HALO + ROFF) * RS]
                bot = t[r0:r1, (HALO + OWN + ROFF) * RS : (HALO + OWN + e + ROFF) * RS]
                nc.vector.tensor_scalar_mul(top, top, MTOP[r0:r1])
                nc.vector.tensor_scalar_mul(bot, bot, MBOT[r0:r1])

            def stream_v(dst, wv_t, src, e):
                """dst[:, rng] = (wv_t.T @ src)[:, rng] via psA, ACT copy evac."""
                for lo, hi in chunks(exr(e), 1024):
                    ps = psA.tile([128, 1024], F32, tag="psA")
                    for l2, h2 in chunks((lo, hi), NT):
                        nc.tensor.matmul(
                            ps[:, l2 - lo : h2 - lo], wv_t[:], src[:, l2:h2],
                            start=True, stop=True,
                        )
                    nc.scalar.copy(dst[:, lo:hi], ps[:, 0 : hi - lo])

            def conv_pe_gelu(dst, src, mats, e_out):
                """dst = gelu(dwconv(src)) over extent e_out, conv on PE."""
                for lo, hi in chunks(exr(e_out), 1024):
                    ps = psB.tile([128, 1024], F32, tag="psB")
                    for l2, h2 in chunks((lo, hi), NT):
                        for t, (dy, dx) in enumerate(TAPS):
                            d = dy * RS + dx
                            nc.tensor.matmul(
                                ps[:, l2 - lo : h2 - lo],
                                mats[:, t * 128 : (t + 1) * 128],
                                src[:, l2 + d : h2 + d],
                                start=(t == 0), stop=(t == 8),
                            )
                    nc.scalar.activation(dst[:, lo:hi], ps[:, 0 : hi - lo], AF.Gelu)

            def conv_dve_gelu(dst, src, coef, e_out):
                """dst = gelu(dwconv(src)) on DVE STT chain, gelu in-place."""
                lo, hi = exr(e_out)
                dy0, dx0 = TAPS[0]
                d0 = dy0 * RS + dx0
                nc.vector.tensor_scalar_mul(
                    dst[:, lo:hi], src[:, lo + d0 : hi + d0], coef[:, 0:1]
                )
                for t in range(1, 9):
                    dy, dx = TAPS[t]
                    d = dy * RS + dx
                    nc.vector.scalar_tensor_tensor(
                        dst[:, lo:hi], src[:, lo + d : hi + d], coef[:, t : t + 1],
                        dst[:, lo:hi], op0=ALU.mult, op1=ALU.add,
                    )
                nc.scalar.activation(dst[:, lo:hi], dst[:, lo:hi], AF.Gelu)

            def conv_dve_acc(dst, src, coef, e_out):
                """dst += dwconv(src): ring-chunk scratch, then one TT add."""
                for lo, hi in chunks(exr(e_out), 1024):
                    scr_ = rp.tile([128, 1024], BF16, tag="cscr", bufs=2)
                    n = hi - lo
                    dy0, dx0 = TAPS[0]
                    d0 = dy0 * RS + dx0
                    nc.vector.tensor_scalar_mul(
                        scr_[:, 0:n], src[:, lo + d0 : hi + d0], coef[:, 0:1]
                    )
                    for t in range(1, 9):
                        dy, dx = TAPS[t]
                        d = dy * RS + dx
                        nc.vector.scalar_tensor_tensor(
                            scr_[:, 0:n], src[:, lo + d : hi + d],
                            coef[:, t : t + 1], scr_[:, 0:n],
                            op0=ALU.mult, op1=ALU.add,
                        )
                    nc.vector.tensor_tensor(
                        dst[:, lo:hi], dst[:, lo:hi], scr_[:, 0:n], op=ALU.add
                    )

            def pair_tail(out_t, v_t, g_t, WoTp, pb_t, resid, pair, e_out, eng2):
                """out_t = out_c(v) [+ dw2(g) if pe] + pb + resid; then dve dw2."""
                mats = load_mdw(f"m_dw2_{pair}") if eng2 == "pe" else None
                for lo, hi in chunks(exr(e_out), 1024):
                    ps = psA.tile([128, 1024], F32, tag="psA")
                    for l2, h2 in chunks((lo, hi), NT):
                        nc.tensor.matmul(
                            ps[:, l2 - lo : h2 - lo], WoTp[:], v_t[:, l2:h2],
                            start=True, stop=(eng2 != "pe"),
                        )
                        if eng2 == "pe":
                            for t, (dy, dx) in enumerate(TAPS):
                                d = dy * RS + dx
                                nc.tensor.matmul(
                                    ps[:, l2 - lo : h2 - lo],
                                    mats[:, t * 128 : (t + 1) * 128],
                                    g_t[:, l2 + d : h2 + d],
                                    start=False, stop=(t == 8),
                                )
                    nc.vector.scalar_tensor_tensor(
                        out_t[:, lo:hi], ps[:, 0 : hi - lo], pb_t[:],
                        resid[:, lo:hi], op0=ALU.add, op1=ALU.add,
                    )
                if eng2 == "dve":
                    conv_dve_acc(out_t, g_t, WT[f"c_dw2_{pair}"], e_out)

            # =============== Grams + AR1 ===============
            arin = dp.tile([128, 4 * 128], F32, tag="arin")
            arout = dp.tile([128, 4 * 128], F32, tag="arout", addr_space="Shared")
            arin_sb = sp.tile([128, 4 * 128], F32, tag="arin_sb")
            g1sb = sp.tile([128, 4 * 128], F32, tag="g1sb")

            for gi, gname in enumerate(["gzx", "gzy"]):
                gps = psC.tile([128, 128], F32, tag="psC")
                nck = OWN * W // 128  # 128 chunks
                for ck in range(nck):
                    t_ = rp.tile([128, 128], BF16, tag="gch", bufs=4)
                    nc.sync.dma_start(t_[:], P[gname][ck * 128 : (ck + 1) * 128, :])
                    nc.tensor.matmul(
                        gps[:], t_[:], t_[:], start=(ck == 0), stop=(ck == nck - 1)
                    )
                nc.vector.tensor_scalar_mul(
                    arin_sb[:, gi * 128 : (gi + 1) * 128], gps[:], SEL0
                )
                nc.vector.tensor_scalar_mul(
                    arin_sb[:, (2 + gi) * 128 : (3 + gi) * 128], gps[:], SEL1
                )
            nc.sync.dma_start(arin[:], arin_sb[:])
            nc.gpsimd.collective_compute(
                "AllReduce", ALU.add, replica_groups=[list(range(8))],
                ins=[arin.opt()], outs=[arout.opt()],
            )
            nc.sync.dma_start(g1sb[:], arout[:])
            # per-batch Gram selection
            gmy = sp.tile([128, 2 * 128], F32, tag="gmy")
            for gi in range(2):
                nc.vector.tensor_scalar_mul(
                    gmy[:, gi * 128 : (gi + 1) * 128],
                    g1sb[:, gi * 128 : (gi + 1) * 128], SEL0,
                )
                nc.vector.scalar_tensor_tensor(
                    gmy[:, gi * 128 : (gi + 1) * 128],
                    g1sb[:, (2 + gi) * 128 : (3 + gi) * 128], SEL1,
                    gmy[:, gi * 128 : (gi + 1) * 128],
                    op0=ALU.mult, op1=ALU.add,
                )

            def attn_module(m, G, qblk, kblk, WoTp, odd):
                """emit small-attn for module m from stack-Gram G [128, 128];
                writes W_oT into WoTp rows/cols r0."""
                r0 = 64 if odd else 0
                wq_s = WT["wqm"][0:62, m * 64 : m * 64 + 62]
                wk_s = WT["wkm"][0:62, m * 64 : m * 64 + 62]
                pw_s = WT["pwm"][0:62, m * 64 : m * 64 + 62]
                q0, k0 = qblk * 64, kblk * 64

                def blk(tag, rblk, cblk):
                    if rblk == 0:
                        return G[0:62, cblk * 64 : cblk * 64 + 62]
                    t_ = sp.tile([128, 64], F32, tag="gblk_" + tag)
                    nc.sync.dma_start(
                        t_[0:62, 0:62], G[64:126, cblk * 64 : cblk * 64 + 62]
                    )
                    return t_[0:62, 0:62]

                gqk = blk("qk", qblk, kblk)
                gqq = blk("qq", qblk, qblk)
                gkk = blk("kk", kblk, kblk)

                def mm62(lhs, rhs, tag):
                    pp = psD.tile([128, 64], F32, tag="psD")
                    nc.tensor.matmul(pp[0:62, 0:62], lhs, rhs, start=True, stop=True)
                    ss = sp.tile([128, 64], F32, tag="t_" + tag, name="t_" + tag)
                    nc.vector.tensor_copy(ss[0:62, 0:62], pp[0:62, 0:62])
                    return ss

                T1 = mm62(gqk, wq_s, "T1")
                T2 = mm62(gqq, wq_s, "T2")
                T3 = mm62(gkk, wk_s, "T3")
                SKQ = mm62(wk_s, T1[0:62, 0:62], "SKQ")
                u2 = sp.tile([128, 64], F32, tag="u2")
                nc.vector.tensor_tensor(
                    u2[0:62, 0:62], wq_s, T2[0:62, 0:62], op=ALU.mult
                )
                u3 = sp.tile([128, 64], F32, tag="u3")
                nc.vector.tensor_tensor(
                    u3[0:62, 0:62], wk_s, T3[0:62, 0:62], op=ALU.mult
                )
                pq = psD.tile([128, 64], F32, tag="psD")
                nc.tensor.matmul(
                    pq[0:1, 0:62], WT["ones62"][0:62, 0:1], u2[0:62, 0:62],
                    start=True, stop=True,
                )
                pk = psD.tile([128, 64], F32, tag="psD")
                nc.tensor.matmul(
                    pk[0:62, 0:1], u3[0:62, 0:62], WT["ones62"][0:62, 0:1],
                    start=True, stop=True,
                )
                ik = sp.tile([128, 2], F32, tag="ik")
                nc.scalar.activation(ik[0:62, 0:1], pk[0:62, 0:1], AF.Sqrt)
                nc.vector.tensor_scalar_max(ik[0:62, 0:1], ik[0:62, 0:1], 1e-12)
                nc.vector.reciprocal(ik[0:62, 1:2], ik[0:62, 0:1])
                scd = sp.tile([128, 1], F32, tag="scd")
                nc.vector.tensor_tensor(
                    scd[0:62, 0:1], ik[0:62, 1:2], WT["rsc"][0:62, m : m + 1],
                    op=ALU.mult,
                )
                iq = sp.tile([1, 128], F32, tag="iq")
                nc.scalar.activation(iq[0:1, 0:62], pq[0:1, 0:62], AF.Sqrt)
                nc.vector.tensor_scalar_max(iq[0:1, 0:62], iq[0:1, 0:62], 1e-12)
                nc.vector.reciprocal(iq[0:1, 64:126], iq[0:1, 0:62])
                iqb = sp.tile([128, 64], F32, tag="iqb")
                nc.gpsimd.partition_broadcast(iqb[0:62, 0:62], iq[0:1, 64:126])
                L = sp.tile([128, 64], F32, tag="L")
                nc.vector.tensor_scalar_mul(L[0:62, 0:62], SKQ[0:62, 0:62], scd[0:62, 0:1])
                nc.vector.tensor_tensor(
                    L[0:62, 0:62], L[0:62, 0:62], iqb[0:62, 0:62], op=ALU.mult
                )
                A = sp.tile([128, 64], F32, tag="A")
                nc.vector.memset(A[:], 0.0)
                nc.vector.tensor_tensor(
                    L[0:62, 0:62], L[0:62, 0:62], WT["hmA"][0:62, 0:62], op=ALU.mult
                )
                nc.vector.tensor_tensor(
                    L[0:62, 0:62], L[0:62, 0:62], WT["hmB"][0:62, 0:62], op=ALU.add
                )
                mx = sp.tile([128, 2], F32, tag="mx")
                nc.vector.tensor_reduce(
                    mx[0:62, 0:1], L[0:62, 0:62], op=ALU.max, axis=mybir.AxisListType.X
                )
                nc.vector.tensor_scalar_mul(mx[0:62, 1:2], mx[0:62, 0:1], -1.0)
                nc.scalar.activation(
                    A[0:62, 0:62], L[0:62, 0:62], AF.Exp, bias=mx[0:62, 1:2]
                )
                sm = sp.tile([128, 2], F32, tag="sm")
                nc.vector.tensor_reduce(
                    sm[0:62, 0:1], A[0:62, 0:62], op=ALU.add, axis=mybir.AxisListType.X
                )
                nc.vector.reciprocal(sm[0:62, 1:2], sm[0:62, 0:1])
                nc.vector.tensor_scalar_mul(A[0:62, 0:62], A[0:62, 0:62], sm[0:62, 1:2])
                wps = psD.tile([128, 64], F32, tag="psD")
                if odd:
                    nc.tensor.matmul(
                        wps[64:126, 0:62], A[0:62, 0:62], pw_s,
                        start=True, stop=True, tile_position=(0, 64),
                    )
                    nc.vector.tensor_copy(WoTp[64:126, 64:126], wps[64:126, 0:62])
                else:
                    nc.tensor.matmul(
                        wps[0:62, 0:62], A[0:62, 0:62], pw_s, start=True, stop=True
                    )
                    nc.vector.tensor_copy(WoTp[0:62, 0:62], wps[0:62, 0:62])

            # WoT pair tiles
            WoT = {}
            for p in PAIRS:
                WoT[p] = sp.tile([128, 128], BF16, tag=f"WoT{p}", name=f"WoT{p}")
                nc.vector.memset(WoT[p][:], 0.0)
            # L1 attn: M0 (gzx stack: q=x blk1, k=x blk1), M1 (gzy: 1,1),
            # M2 (gzx: q=x 1, k=z 0), M3 (gzy: 1, 0)
            attn_module(0, gmy[:, 0:128], 1, 1, WoT["01"], odd=False)
            attn_module(1, gmy[:, 128:256], 1, 1, WoT["01"], odd=True)
            attn_module(2, gmy[:, 0:128], 1, 0, WoT["23"], odd=False)
            attn_module(3, gmy[:, 128:256], 1, 0, WoT["23"], odd=True)

            # =============== load slabs ===============
            xy = bp.tile([128, FA], BF16, tag="xy")
            nc.sync.dma_start(xy[:, RS : RS + R * RS], P["xy"][:])
            zz = bp.tile([128, FA], BF16, tag="s1")
            nc.sync.dma_start(zz[:, RS : RS + R * RS], P["zz"][:])

            # =============== P23: modules 2,3 ===============
            v23 = bp.tile([128, FA], BF16, tag="s2")
            stream_v(v23, WT["wv23"], zz, 6)
            pad_zero(v23)
            g23 = bp.tile([128, FA], BF16, tag="s3")
            if ENG["dw1_23"] == "pe":
                conv_pe_gelu(g23, v23, load_mdw("m_dw1_23"), 5)
            else:
                conv_dve_gelu(g23, v23, WT["c_dw1_23"], 5)
            pad_zero(g23)
            mask_rows(g23, 5)
            fafb = bp.tile([128, FA], BF16, tag="s1")  # reuses zz slot
            pair_tail(fafb, v23, g23, WoT["23"], WT["pb23"], xy, "23", 4,
                      ENG["dw2_23"])
            pad_zero(fafb)
            mask_rows(fafb, 4)
            # spill fa|fb to DRAM
            fafb_d = dp.tile([128, R * RS], BF16, tag="fafb_d")
            nc.sync.dma_start(fafb_d[:], fafb[:, RS : RS + R * RS])

            # =============== P01: modules 0,1 ===============
            v01 = bp.tile([128, FA], BF16, tag="s2")
            stream_v(v01, WT["wv01"], xy, 4)
            pad_zero(v01)
            g01 = bp.tile([128, FA], BF16, tag="s3")
            if ENG["dw1_01"] == "pe":
                conv_pe_gelu(g01, v01, load_mdw("m_dw1_01"), 3)
            else:
                conv_dve_gelu(g01, v01, WT["c_dw1_01"], 3)
            pad_zero(g01)
            mask_rows(g01, 3)
            xaYb = bp.tile([128, FA], BF16, tag="s1")
            pair_tail(xaYb, v01, g01, WoT["01"], WT["pb01"], xy, "01", 2,
                      ENG["dw2_01"])
            pad_zero(xaYb)
            mask_rows(xaYb, 2)

            # =============== L2 Grams + AR2 ===============
            fafb2 = bp.tile([128, FA], BF16, tag="s3")
            nc.sync.dma_start(fafb2[:, RS : RS + R * RS], fafb_d[:])
            v45 = bp.tile([128, FA], BF16, tag="s2")
            stream_v(v45, WT["wv45"], fafb2, 4)
            pad_zero(v45)
            arin2_sb = sp.tile([128, 4 * 128], F32, tag="arin_sb")
            for gi, (r0,) in enumerate([(0,), (64,)]):  # chain A, chain B
                gps = psC.tile([128, 128], F32, tag="psC")
                nck = OWN * RS // 128  # 129 chunks (incl pad cols = zeros)
                for ck in range(nck):
                    off = (HALO + ROFF) * RS + ck * 128
                    t_ = rp.tile([128, 128], BF16, tag="gch", bufs=4)
                    nc.sync.dma_start(
                        t_[:, 0:64], xaYb[r0 : r0 + 64, off : off + 128],
                        transpose=True,
                    )
                    nc.sync.dma_start(
                        t_[:, 64:128], fafb2[r0 : r0 + 64, off : off + 128],
                        transpose=True,
                    )
                    nc.tensor.matmul(
                        gps[:], t_[:], t_[:], start=(ck == 0), stop=(ck == nck - 1)
                    )
                nc.vector.tensor_scalar_mul(
                    arin2_sb[:, gi * 128 : (gi + 1) * 128], gps[:], SEL0
                )
                nc.vector.tensor_scalar_mul(
                    arin2_sb[:, (2 + gi) * 128 : (3 + gi) * 128], gps[:], SEL1
                )
            arin2 = dp.tile([128, 4 * 128], F32, tag="arin2")
            arout2 = dp.tile([128, 4 * 128], F32, tag="arout2", addr_space="Shared")
            nc.sync.dma_start(arin2[:], arin2_sb[:])
            nc.gpsimd.collective_compute(
                "AllReduce", ALU.add, replica_groups=[list(range(8))],
                ins=[arin2.opt()], outs=[arout2.opt()],
            )
            g2sb = sp.tile([128, 4 * 128], F32, tag="g1sb")
            nc.sync.dma_start(g2sb[:], arout2[:])
            gmy2 = sp.tile([128, 2 * 128], F32, tag="gmy")
            for gi in range(2):
                nc.vector.tensor_scalar_mul(
                    gmy2[:, gi * 128 : (gi + 1) * 128],
                    g2sb[:, gi * 128 : (gi + 1) * 128], SEL0,
                )
                nc.vector.scalar_tensor_tensor(
                    gmy2[:, gi * 128 : (gi + 1) * 128],
                    g2sb[:, (2 + gi) * 128 : (3 + gi) * 128], SEL1,
                    gmy2[:, gi * 128 : (gi + 1) * 128],
                    op0=ALU.mult, op1=ALU.add,
                )
            # stacks are [xa|fa]: q=xa blk0, k=fa blk1
            attn_module(4, gmy2[:, 0:128], 0, 1, WoT["45"], odd=False)
            attn_module(5, gmy2[:, 128:256], 0, 1, WoT["45"], odd=True)

            # =============== P45 ===============
            g45 = bp.tile([128, FA], BF16, tag="s3")
            if ENG["dw1_45"] == "pe":
                conv_pe_gelu(g45, v45, load_mdw("m_dw1_45"), 3)
            else:
                conv_dve_gelu(g45, v45, WT["c_dw1_45"], 3)
            pad_zero(g45)
            mask_rows(g45, 3)
            t12 = bp.tile([128, FA], BF16, tag="xy")  # xy slot long dead
            pair_tail(t12, v45, g45, WoT["45"], WT["pb45"], xaYb, "45", 2,
                      ENG["dw2_45"])
            pad_zero(t12)
            mask_rows(t12, 2)

            # =============== LN + FF (per chain) ===============
            # spill xa|yb during FF
            xaYb_d = dp.tile([128, R * RS], BF16, tag="xaYb_d")
            nc.sync.dma_start(xaYb_d[:], xaYb[:, RS : RS + R * RS])

            lo2, hi2 = exr(2)
            lo1, hi1 = exr(1)
            L2p = 64 * 275  # 17600 staging per stat row
            stat_all_d = dp.tile([2, 2 * L2p], F32, tag="stat_all_d")
            row_d = dp.tile([1, 2 * L2p], BF16, tag="row_d")

            # stats for both chains at once, streamed to DRAM
            sq = bp.tile([128, FA], BF16, tag="s2")  # reuses v45 slot
            nc.scalar.activation(sq[:, lo2:hi2], t12[:, lo2:hi2], AF.Square)
            for si, srct in enumerate([t12, sq]):
                base = si * L2LEN
                for lo, hi in chunks((lo2, hi2), 512):
                    ps = psA.tile([128, 1024], F32, tag="psA")
                    nc.tensor.matmul(
                        ps[0:2, 0 : hi - lo], WT["lnones"][:],
                        srct[:, lo:hi], start=True, stop=True,
                    )
                    stch = rp.tile([2, 512], F32, tag="stch", bufs=2)
                    nc.scalar.copy(stch[:, 0 : hi - lo], ps[0:2, 0 : hi - lo])
                    nc.sync.dma_start(
                        stat_all_d[:, base + lo - lo2 : base + hi - lo2],
                        stch[:, 0 : hi - lo],
                    )

            for ci, cn in ((0, "a"), (1, "b")):
                r0 = 0 if ci == 0 else 64
                # reshape stats row ci -> [64, 275] domain
                mu64 = sp.tile([64, 275], F32, tag="mu64")
                ms64 = sp.tile([64, 275], F32, tag="ms64")
                nc.sync.dma_start(
                    mu64[:],
                    stat_all_d[ci : ci + 1, 0:L2p].rearrange(
                        "o (p f) -> (o p) f", p=64
                    ),
                )
                nc.sync.dma_start(
                    ms64[:],
                    stat_all_d[ci : ci + 1, L2LEN : L2LEN + L2p].rearrange(
                        "o (p f) -> (o p) f", p=64
                    ),
                )
                var = sp.tile([64, 275], F32, tag="var64")
                nc.vector.tensor_tensor(var[:], mu64[:], mu64[:], op=ALU.mult)
                nc.vector.tensor_tensor(var[:], ms64[:], var[:], op=ALU.subtract)
                sd = sp.tile([64, 275], F32, tag="sd64")
                nc.vector.tensor_scalar_add(var[:], var[:], 1e-5)
                nc.scalar.activation(sd[:], var[:], AF.Sqrt)
                inv = sp.tile([64, 275], F32, tag="inv64")
                nc.vector.reciprocal(inv[:], sd[:])
                invb = sp.tile([64, 275], BF16, tag="invb64")
                nc.vector.tensor_copy(invb[:], inv[:])
                musb = sp.tile([64, 275], BF16, tag="musb64")
                nc.vector.tensor_tensor(var[:], mu64[:], inv[:], op=ALU.mult)
                nc.vector.tensor_copy(musb[:], var[:])
                # back to rows in DRAM
                nc.sync.dma_start(
                    row_d[0:1, 0:L2p].rearrange("o (p f) -> (o p) f", p=64), invb[:]
                )
                nc.sync.dma_start(
                    row_d[0:1, L2p : 2 * L2p].rearrange("o (p f) -> (o p) f", p=64),
                    musb[:],
                )
                # broadcast s-hat, build ts
                sB = bp.tile([128, FA], BF16, tag="s2")
                nc.sync.dma_start(
                    sB[:, 0:L2LEN], row_d[0:1, 0:L2LEN].partition_broadcast(128)
                )
                ts = bp.tile([128, FA], BF16, tag="s1")
                nc.vector.tensor_tensor(
                    ts[r0 : r0 + 62, lo2:hi2], t12[r0 : r0 + 62, lo2:hi2],
                    sB[r0 : r0 + 62, 0:L2LEN], op=ALU.mult,
                )
                # mu*s row into augmentation row r0+62
                nc.sync.dma_start(
                    ts[r0 + 62 : r0 + 63, lo2:hi2], row_d[0:1, L2p : L2p + L2LEN]
                )
                th = {}
                for h, hn in ((0, "0"), (1, "1")):
                    tht = bp.tile([128, FA], BF16, tag=("s3" if h == 0 else "s2"))
                    th[h] = tht  # named via tht
                    w1t = WT[f"w1aug_{cn}{hn}"]
                    c1t = WT[f"c1b_{cn}{hn}"]
                    for lo, hi in chunks((lo2, hi2), 1024):
                        ps = psA.tile([128, 1024], F32, tag="psA")
                        for l2, h2 in chunks((lo, hi), NT):
                            nc.tensor.matmul(
                                ps[0:124, l2 - lo : h2 - lo], w1t[:], ts[:, l2:h2],
                                start=True, stop=True,
                            )
                        nc.scalar.activation(
                            tht[0:124, lo:hi], ps[0:124, 0 : hi - lo], AF.Gelu,
                            bias=c1t[0:124, :],
                        )
                    pad_zero(tht)
                    mask_rows(tht, 2, (0, 124))
                # conv h0/h1 + gelu + w2, chunk-wise
                for lo, hi in chunks((lo1, hi1), 1024):
                    wps = psA.tile([128, 1024], F32, tag="psA")
                    for h, hn in ((0, "0"), (1, "1")):
                        eng = ENG[f"ff_{cn}{hn}"]
                        ghc = rp.tile([128, 1024], BF16, tag="ghc", bufs=3)
                        if eng == "pe":
                            cps = psB.tile([128, 1024], F32, tag="psB")
                            mats = load_mdw(f"m_ffdw_{cn}{hn}")
                            for l2, h2 in chunks((lo, hi), NT):
                                for t, (dy, dx) in enumerate(TAPS):
                                    d = dy * RS + dx
                                    nc.tensor.matmul(
                                        cps[:, l2 - lo : h2 - lo],
                                        mats[:, t * 128 : (t + 1) * 128],
                                        th[h][:, l2 + d : h2 + d],
                                        start=(t == 0), stop=(t == 8),
                                    )
                            nc.scalar.activation(
                                ghc[0:124, 0 : hi - lo], cps[0:124, 0 : hi - lo],
                                AF.Gelu,
                            )
                        else:
                            coef = WT[f"c_ffdw_{cn}{hn}"]
                            dy0, dx0 = TAPS[0]
                            d0 = dy0 * RS + dx0
                            nc.vector.tensor_scalar_mul(
                                ghc[0:124, 0 : hi - lo],
                                th[h][0:124, lo + d0 : hi + d0], coef[0:124, 0:1],
                            )
                            for t in range(1, 9):
                                dy, dx = TAPS[t]
                                d = dy * RS + dx
                                nc.vector.scalar_tensor_tensor(
                                    ghc[0:124, 0 : hi - lo],
                                    th[h][0:124, lo + d : hi + d],
                                    coef[0:124, t : t + 1],
                                    ghc[0:124, 0 : hi - lo],
                                    op0=ALU.mult, op1=ALU.add,
                                )
                            nc.scalar.activation(
                                ghc[0:124, 0 : hi - lo], ghc[0:124, 0 : hi - lo],
                                AF.Gelu,
                            )
                        w2t = WT[f"w2h_{cn}{hn}"]
                        for l2, h2 in chunks((lo, hi), NT):
                            nc.tensor.matmul(
                                wps[r0 : r0 + 62, l2 - lo : h2 - lo],
                                w2t[0:124, 0:62],
                                ghc[0:124, l2 - lo : h2 - lo],
                                start=(h == 0), stop=(h == 1),
                                tile_position=(0, 64) if ci == 1 else None,
                            )
                    # t12 += w2 out (in place, one rounding)
                    nc.vector.scalar_tensor_tensor(
                        t12[r0 : r0 + 62, lo:hi], wps[r0 : r0 + 62, 0 : hi - lo],
                        1.0, t12[r0 : r0 + 62, lo:hi], op0=ALU.mult, op1=ALU.add,
                    )

            # =============== fusions + final ===============
            xaYb2 = bp.tile([128, FA], BF16, tag="s1")
            nc.sync.dma_start(xaYb2[:, RS : RS + R * RS], xaYb_d[:])
            fu1 = bp.tile([128, FA], BF16, tag="s2")
            fu2 = bp.tile([128, FA], BF16, tag="s3")
            nc.vector.tensor_copy(fu1[0:62, lo1:hi1], t12[0:62, lo1:hi1])
            nc.sync.dma_start(fu1[64:126, lo1:hi1], xaYb2[0:62, lo1:hi1])
            nc.sync.dma_start(fu2[0:62, lo1:hi1], t12[64:126, lo1:hi1])
            nc.vector.tensor_copy(fu2[64:126, lo1:hi1], xaYb2[64:126, lo1:hi1])
            fab = bp.tile([128, FA], BF16, tag="xy")  # xy slot long dead
            for lo, hi in chunks((lo1, hi1), 1024):
                ps = psA.tile([128, 1024], F32, tag="psA")
                for l2, h2 in chunks((lo, hi), NT):
                    nc.tensor.matmul(
                        ps[:, l2 - lo : h2 - lo], WT["fu1w"][:], fu1[:, l2:h2],
                        start=True, stop=False,
                    )
                    nc.tensor.matmul(
                        ps[:, l2 - lo : h2 - lo], WT["fu2w"][:], fu2[:, l2:h2],
                        start=False, stop=True,
                    )
                nc.scalar.add(fab[:, lo:hi], ps[:, 0 : hi - lo], WT["fucb"][:])
            pad_zero(fab)
            mask_rows(fab, 1)
            fin = bp.tile([128, FA], BF16, tag="s1")
            for lo, hi in chunks((lo1, hi1), 1024):
                ps = psA.tile([128, 1024], F32, tag="psA")
                for l2, h2 in chunks((lo, hi), NT):
                    nc.tensor.matmul(
                        ps[0:64, l2 - lo : h2 - lo], WT["outw"][:], fab[:, l2:h2],
                        start=True, stop=True,
                    )
                nc.scalar.add(fin[0:62, lo:hi], ps[0:62, 0 : hi - lo], WT["outb"][0:62, :])
            nc.sync.dma_start(fin[64:126, lo1:hi1], P["zc"][0:62, :])
            pad_zero(fin)
            mask_rows(fin, 1, (0, 62))
            out_stage = dp.tile([62, OWN * RS], F32, tag="out_stage")
            lo0, hi0 = exr(0)
            for lo, hi in chunks((lo0, hi0), 1024):
                ps = psA.tile([128, 1024], F32, tag="psA")
                for l2, h2 in chunks((lo, hi), NT):
                    for t in range(9):
                        dy, dx = TAPS[t]
                        d = dy * RS + dx
                        nc.tensor.matmul(
                            ps[0:64, l2 - lo : h2 - lo],
                            WT["finw"][:, t * 64 : (t + 1) * 64],
                            fin[:, l2 + d : h2 + d],
                            start=(t == 0), stop=(t == 8),
                        )
                for l3, h3 in chunks((lo, hi), 512):
                    och = rp.tile([62, 512], F32, tag="och", bufs=2)
                    nc.scalar.add(och[:, 0 : h3 - l3], ps[0:62, l3 - lo : h3 - lo], WT["finb"][0:62, :])
                    nc.sync.dma_start(
                        out_stage[:, l3 - lo0 : h3 - lo0], och[:, 0 : h3 - l3]
                    )
            nc.sync.dma_start(
                out_p[:].rearrange("c (r w) -> c r w", w=W),
                out_stage[:].rearrange("c (r s) -> c r s", s=RS)[:, :, 0:W],
            )

    nc.finalize()
    return nc


_NC_CACHE = {}


def _run(inputs, trace=False):
    if "nc" not in _NC_CACHE:
        _NC_CACHE["nc"] = build_nc()
    nc = _NC_CACHE["nc"]
    names = {
        a.name.removesuffix("_set")
        for a in nc.m.functions[0].allocations
        if getattr(a, "kind", None) == "ExternalInput"
    }
    in_maps = prep_host_inputs(inputs)
    in_maps = [{k: v for k, v in m.items() if k in names} for m in in_maps]
    res = run_bass_kernel_spmd(
        nc, in_maps, core_ids=list(range(8)), trace=trace
    )
    return assemble_output(res.results), res


def kernel(**inputs):
    out, _ = _run(inputs, trace=False)
    return out



# revision 15
# speedup vs baseline: 1.8595x; 1.8595x over previous
"""Bass/Trainium2 kernel for nn_Block_14010183320003 (MST++-style block).

Sharding: 8 cores = 2 batches x 4 row-quarters (64 owned rows each, halo 6).
Chain packing: partitions 0:62 = x-derived chain A, 64:126 = y-derived chain B.
Attention is computed from 124-dim input Gram matrices (no q/k tensors);
per-core partial Grams are summed with per-batch-group (4-core) AllReduces.
Row layout: stride 258 per row (2 zero pad cols) so 3x3 convs are free-axis
shifted reads; depthwise convs run as PE diag-matmul accumulation or DVE
scalar_tensor_tensor chains per engine flags.

v2: L2 Gram via PE transposes (was serial DMA transposes), conv work
reordered ahead of attention so the AllReduce skew/latency hides under PE
work, fusion as accumulating matmuls (no SBUF concat copies), 1024-col bf16
moving chunks, chunk-interleaved LN stats, 2KB-line Gram1 stack layout.
"""
import sys

sys.path.insert(0, "/opt/trn_rl_repo")
import numpy as np
import ml_dtypes

import concourse.bass as bass
import concourse.mybir as mybir
import concourse.tile as tile
import concourse.bacc as bacc
from concourse.bass_utils import run_bass_kernel_spmd

BF16, F32 = mybir.dt.bfloat16, mybir.dt.float32
AF = mybir.ActivationFunctionType
ALU = mybir.AluOpType
bf = ml_dtypes.bfloat16

C = 62
W = 256
RS = 258  # row stride (2 zero pad cols)
OWN = 64
HALO = 6
R = 76
ROFF = 1  # lead pad row at tile row 0 (zero pad for shifted reads)
RA = 78  # lead pad + 76 slab rows + 1 trailing spare
FA = RA * RS  # 20124
NT = 1024  # matmul moving chunk (bf16 allows 1024)
NH, DH = 2, 31

# conv engine flags per site: "pe" or "dve"
ENG = {
    "dw1_01": "pe", "dw2_01": "pe",
    "dw1_23": "pe", "dw2_23": "pe",
    "dw1_45": "pe", "dw2_45": "pe",
    "ff_a0": "pe", "ff_a1": "pe", "ff_b0": "pe", "ff_b1": "pe",
}

TAPS = [(dy, dx) for dy in (-1, 0, 1) for dx in (-1, 0, 1)]


def exr(e):
    """free range of extent e (slab rows [6-e, 70+e)), incl lead-row offset."""
    return ((HALO - e + ROFF) * RS, (HALO + OWN + e + ROFF) * RS)


def _slab(full, row0):
    """full [C, 256, 256] -> [C, R, RS] zero-padded slab (rows row0..row0+R)."""
    out = np.zeros((C, R, RS), np.float32)
    lo, hi = max(0, row0), min(256, row0 + R)
    out[:, lo - row0 : hi - row0, :W] = full[:, lo:hi]
    return out


def _to_bf(a):
    return np.ascontiguousarray(a.astype(bf))


def _diag_taps(kvec_by_tap):
    """kvec_by_tap: [9, 128] -> [128, 9*128] bf16 block of diag matrices."""
    out = np.zeros((128, 9 * 128), np.float32)
    for t in range(9):
        out[:, t * 128 : (t + 1) * 128][np.arange(128), np.arange(128)] = kvec_by_tap[t]
    return _to_bf(out)


def _pair128(a_block, b_block):
    """[62, m] + [62, m] -> [128, 128] block-diag lhsT."""
    out = np.zeros((128, 128), np.float32)
    out[0:62, 0:62] = a_block
    out[64:126, 64:126] = b_block
    return _to_bf(out)


def prep_host_inputs(inputs):
    """Build the 8 per-core input maps."""
    inp = {k: np.asarray(v, dtype=np.float32) for k, v in inputs.items()}
    wq, wk, wv = inp["attn_wq"], inp["attn_wk"], inp["attn_wv"]
    pw, pb = inp["attn_pw"], inp["attn_pb"]
    dw1, dw2 = inp["attn_dw1"], inp["attn_dw2"]
    resc = inp["attn_rescale"]

    shared = {}
    for pair, (me, mo) in {"01": (0, 1), "23": (2, 3), "45": (4, 5)}.items():
        shared[f"wv{pair}"] = _pair128(wv[me], wv[mo])
        pbv = np.zeros((128, 1), np.float32)
        pbv[0:62, 0] = pb[me]
        pbv[64:126, 0] = pb[mo]
        shared[f"pb{pair}"] = pbv
        for cn, dwk in (("1", dw1), ("2", dw2)):
            kc = np.zeros((128, 9), np.float32)
            for t, (dy, dx) in enumerate(TAPS):
                kc[0:62, t] = dwk[me][:, dy + 1, dx + 1]
                kc[64:126, t] = dwk[mo][:, dy + 1, dx + 1]
            shared[f"c_dw{cn}_{pair}"] = kc
            shared[f"m_dw{cn}_{pair}"] = _diag_taps(kc.T)
    # attn small weights: [128, 6*64] col block m at m*64, rows 0:62
    for nm, src in (("wqm", wq), ("wkm", wk), ("pwm", pw)):
        t = np.zeros((128, 6 * 64), np.float32)
        for m in range(6):
            t[0:62, m * 64 : m * 64 + 62] = src[m]
        shared[nm] = t
    rs = np.zeros((128, 6), np.float32)
    for m in range(6):
        rs[0:62, m] = np.repeat(resc[m], DH)
    shared["rsc"] = rs
    ones62 = np.zeros((128, 1), np.float32)
    ones62[0:62, 0] = 1.0
    shared["ones62"] = ones62
    hmA = np.zeros((128, 64), np.float32)
    hmB = np.full((128, 64), -1e4, np.float32)
    for h in range(NH):
        hmA[h * DH : (h + 1) * DH, h * DH : (h + 1) * DH] = 1.0
        hmB[h * DH : (h + 1) * DH, h * DH : (h + 1) * DH] = 0.0
    shared["hmA"] = hmA
    shared["hmB"] = hmB
    lnones = np.zeros((128, 2), np.float32)
    lnones[0:62, 0] = 1.0 / C
    lnones[64:126, 1] = 1.0 / C
    shared["lnones"] = _to_bf(lnones)
    # identity for PE transposes: eye64 in rows 0:64 and rows 64:128
    id2 = np.zeros((128, 64), np.float32)
    id2[0:64, 0:64] = np.eye(64)
    id2[64:128, 0:64] = np.eye(64)
    shared["id64"] = _to_bf(id2)

    # FF weights. chain A uses ff index 0, chain B index 1.
    for ci, cname in ((0, "a"), (1, "b")):
        g, b = inp["ln_g"][ci], inp["ln_b"][ci]
        w1 = inp["ff_w1"][ci]  # [62, 248]
        w2 = inp["ff_w2"][ci]  # [248, 62]
        w1p = g[:, None] * w1
        c2 = w1p.sum(axis=0)  # [248]
        c1 = b @ w1  # [248]
        dwk = inp["ff_dw"][ci]  # [248, 3, 3]
        r0 = 0 if ci == 0 else 64
        for h in (0, 1):
            sl = slice(h * 124, (h + 1) * 124)
            wt = np.zeros((128, 124), np.float32)
            wt[r0 : r0 + 62, :] = w1p[:, sl]
            wt[r0 + 62, :] = -c2[sl]
            shared[f"w1aug_{cname}{h}"] = _to_bf(wt)
            cb = np.zeros((128, 1), np.float32)
            cb[0:124, 0] = c1[sl]
            shared[f"c1b_{cname}{h}"] = cb
            kc = np.zeros((128, 9), np.float32)
            for t, (dy, dx) in enumerate(TAPS):
                kc[0:124, t] = dwk[sl, dy + 1, dx + 1]
            shared[f"c_ffdw_{cname}{h}"] = kc
            shared[f"m_ffdw_{cname}{h}"] = _diag_taps(kc.T)
            w2t = np.zeros((128, 128), np.float32)
            w2t[0:124, 0:62] = w2[sl, :]
            shared[f"w2h_{cname}{h}"] = _to_bf(w2t)

    # fusion weights: fa2 = fuc1_w.T @ [t1f; xa], fb2 = fuc2_w.T @ [t2f; yb]
    # as two accumulating matmuls: fuT acts on t12 (chains packed), fuX on xaYb
    fuT = np.zeros((128, 128), np.float32)
    fuT[0:62, 0:62] = inp["fuc1_w"][:, 0:62].T
    fuT[64:126, 64:126] = inp["fuc2_w"][:, 0:62].T
    shared["fuT"] = _to_bf(fuT)
    fuX = np.zeros((128, 128), np.float32)
    fuX[0:62, 0:62] = inp["fuc1_w"][:, 62:124].T
    fuX[64:126, 64:126] = inp["fuc2_w"][:, 62:124].T
    shared["fuX"] = _to_bf(fuX)
    fucb = np.zeros((128, 1), np.float32)
    fucb[0:62, 0] = inp["fuc1_b"]
    fucb[64:126, 0] = inp["fuc2_b"]
    shared["fucb"] = fucb
    ow = np.zeros((128, 64), np.float32)
    ow[0:62, 0:62] = inp["outc_w"][:, 0:62].T
    ow[64:126, 0:62] = inp["outc_w"][:, 62:124].T
    shared["outw"] = _to_bf(ow)
    ob = np.zeros((128, 1), np.float32)
    ob[0:62, 0] = inp["outc_b"]
    shared["outb"] = ob
    fw = np.zeros((128, 9 * 64), np.float32)
    for t, (dy, dx) in enumerate(TAPS):
        fw[0:62, t * 64 : t * 64 + 62] = inp["final_w"][:, 0:62, dy + 1, dx + 1].T
        fw[64:126, t * 64 : t * 64 + 62] = inp["final_w"][:, 62:124, dy + 1, dx + 1].T
    shared["finw"] = _to_bf(fw)
    fb_ = np.zeros((128, 1), np.float32)
    fb_[0:62, 0] = inp["final_b"]
    shared["finb"] = fb_

    for k in list(shared.keys()):
        if shared[k].dtype == np.float32:
            shared[k] = np.ascontiguousarray(shared[k])

    in_maps = []
    for b in range(2):
        xb, yb_, zb = inp["x"][b], inp["y"][b], inp["z"][b]
        for s in range(4):
            row0 = 64 * s - HALO
            xs, ys, zs = _slab(xb, row0), _slab(yb_, row0), _slab(zb, row0)
            m = {}
            xy = np.zeros((128, R, RS), np.float32)
            xy[0:62], xy[64:126] = xs, ys
            m["xy"] = _to_bf(xy.reshape(128, R * RS))
            zz = np.zeros((128, R, RS), np.float32)
            zz[0:62], zz[64:126] = zs, zs
            m["zz"] = _to_bf(zz.reshape(128, R * RS))
            m["zc"] = _to_bf(
                np.concatenate([zs[:, 5:71].reshape(62, 66 * RS),
                                np.zeros((66, 66 * RS), np.float32)], 0)
            )
            # pixel-major Gram stacks over owned real pixels, laid out so each
            # [128, 1024] DMA line holds 8 consecutive 128-pixel lhsT chunks:
            # h[s, p, k, :] = g[s*1024 + k*128 + p, :]
            def pixmaj(t1, t2):
                g = np.zeros((OWN * W, 128), np.float32)
                g[:, 0:62] = t1[:, HALO : HALO + OWN, :W].reshape(C, -1).T
                g[:, 64:126] = t2[:, HALO : HALO + OWN, :W].reshape(C, -1).T
                h = g.reshape(16, 8, 128, 128).transpose(0, 2, 1, 3)
                return _to_bf(np.ascontiguousarray(h.reshape(16 * 128, 8 * 128)))
            m["gzx"] = pixmaj(zs, xs)
            m["gzy"] = pixmaj(zs, ys)
            msk = np.zeros((128, 4), np.float32)
            msk[:, 0] = 0.0 if s == 0 else 1.0
            msk[:, 1] = 0.0 if s == 3 else 1.0
            msk[:, 2] = 1.0 if b == 0 else 0.0
            msk[:, 3] = 1.0 if b == 1 else 0.0
            m["msk"] = msk
            m.update(shared)
            in_maps.append(m)
    return in_maps


def assemble_output(results):
    out = np.zeros((2, C, 256, 256), np.float32)
    for b in range(2):
        for s in range(4):
            r = results[b * 4 + s]["out"]  # [62, 64*256]
            out[b, :, 64 * s : 64 * (s + 1), :] = r.reshape(C, OWN, W)
    return out


# ---------------------------------------------------------------------------
# device IR
# ---------------------------------------------------------------------------

PAIRS = ["01", "23", "45"]
L2LEN = exr(2)[1] - exr(2)[0]  # 17544
GROUPS = [[0, 1, 2, 3], [4, 5, 6, 7]]  # per-batch AllReduce groups


def build_nc():
    nc = bacc.Bacc(None, target_bir_lowering=False, debug=False)

    P = {}
    for nm in ["xy", "zz"]:
        P[nm] = nc.declare_dram_parameter(nm, [128, R * RS], BF16, isOutput=False)
    P["zc"] = nc.declare_dram_parameter("zc", [128, 66 * RS], BF16, isOutput=False)
    P["gzx"] = nc.declare_dram_parameter("gzx", [16 * 128, 8 * 128], BF16, isOutput=False)
    P["gzy"] = nc.declare_dram_parameter("gzy", [16 * 128, 8 * 128], BF16, isOutput=False)
    P["msk"] = nc.declare_dram_parameter("msk", [128, 4], F32, isOutput=False)
    wnames_bf = (
        [f"wv{p}" for p in PAIRS]
        + ["lnones", "id64"]
        + [f"w1aug_{c}{h}" for c in "ab" for h in "01"]
        + [f"w2h_{c}{h}" for c in "ab" for h in "01"]
        + ["fuT", "fuX", "outw", "finw"]
    )
    wnames_f32 = (
        [f"pb{p}" for p in PAIRS]
        + [f"c_dw{c}_{p}" for c in "12" for p in PAIRS]
        + ["rsc", "wqm", "wkm", "pwm", "ones62", "hmA", "hmB"]
        + [f"c_ffdw_{c}{h}" for c in "ab" for h in "01"]
        + [f"c1b_{c}{h}" for c in "ab" for h in "01"]
        + ["fucb", "outb", "finb"]
    )
    WSHAPE = {
        "wqm": [128, 6 * 64], "wkm": [128, 6 * 64], "pwm": [128, 6 * 64],
        "ones62": [128, 1], "lnones": [128, 2], "rsc": [128, 6],
        "outw": [128, 64], "finw": [128, 9 * 64],
        "hmA": [128, 64], "hmB": [128, 64],
        "fuT": [128, 128], "fuX": [128, 128], "id64": [128, 64],
        "fucb": [128, 1], "outb": [128, 1], "finb": [128, 1],
    }
    for p in PAIRS:
        WSHAPE[f"wv{p}"] = [128, 128]
        WSHAPE[f"pb{p}"] = [128, 1]
        for c in "12":
            WSHAPE[f"m_dw{c}_{p}"] = [128, 9 * 128]
            WSHAPE[f"c_dw{c}_{p}"] = [128, 9]
    for c in "ab":
        for h in "01":
            WSHAPE[f"w1aug_{c}{h}"] = [128, 124]
            WSHAPE[f"m_ffdw_{c}{h}"] = [128, 9 * 128]
            WSHAPE[f"c_ffdw_{c}{h}"] = [128, 9]
            WSHAPE[f"c1b_{c}{h}"] = [128, 1]
            WSHAPE[f"w2h_{c}{h}"] = [128, 128]
    mnames = [f"m_dw{c}_{p}" for c in "12" for p in PAIRS
              if ENG[f"dw{c}_{p}"] == "pe"] + [
        f"m_ffdw_{c}{h}" for c in "ab" for h in "01" if ENG[f"ff_{c}{h}"] == "pe"
    ]
    for nm in wnames_bf + mnames:
        P[nm] = nc.declare_dram_parameter(nm, WSHAPE[nm], BF16, isOutput=False)
    for nm in wnames_f32:
        P[nm] = nc.declare_dram_parameter(nm, WSHAPE[nm], F32, isOutput=False)
    out_p = nc.declare_dram_parameter("out", [C, OWN * W], F32, isOutput=True)

    with tile.TileContext(nc, num_cores=8) as tc:
        with (
            tc.tile_pool(name="w", bufs=1) as wp,
            tc.tile_pool(name="small", bufs=1) as sp,
            tc.tile_pool(name="big", bufs=1) as bp,
            tc.tile_pool(name="ring", bufs=3) as rp,
            tc.tile_pool(name="psA", bufs=2, space="PSUM") as psA,
            tc.tile_pool(name="psB", bufs=1, space="PSUM") as psB,
            tc.tile_pool(name="psC", bufs=1, space="PSUM") as psC,
            tc.tile_pool(name="psD", bufs=1, space="PSUM") as psD,
            tc.tile_pool(name="dram", bufs=1, space="DRAM") as dp,
        ):
            msk = sp.tile([128, 4], F32, tag="msk")
            nc.sync.dma_start(msk[:], P["msk"][:])
            MTOP, MBOT = msk[:, 0:1], msk[:, 1:2]
            WT = {}
            for nm in wnames_bf:
                WT[nm] = wp.tile(WSHAPE[nm], BF16, tag=nm, name=nm)
                nc.sync.dma_start(WT[nm][:], P[nm][:])
            for nm in wnames_f32:
                WT[nm] = wp.tile(WSHAPE[nm], F32, tag=nm, name=nm)
                nc.sync.dma_start(WT[nm][:], P[nm][:])

            def load_mdw(nm):
                t_ = rp.tile([128, 9 * 128], BF16, tag="mdw", bufs=2, name=nm + "_l")
                nc.sync.dma_start(t_[:], P[nm][:])
                return t_

            def chunks(rng, step):
                lo, hi = rng
                out = []
                while lo < hi:
                    out.append((lo, min(lo + step, hi)))
                    lo += step
                return out

            def pad_zero(t, lo_row=0, hi_row=RA):
                v = t[:].rearrange("p (r s) -> p r s", s=RS)[:, lo_row:hi_row, W : W + 2]
                nc.vector.memset(v, 0.0)

            def mask_rows(t, e, dtype_rows=(0, 128)):
                if e <= 0:
                    return
                r0, r1 = dtype_rows
                top = t[r0:r1, (HALO - e + ROFF) * RS : (HALO + ROFF) * RS]
                bot = t[r0:r1, (HALO + OWN + ROFF) * RS : (HALO + OWN + e + ROFF) * RS]
                nc.vector.tensor_scalar_mul(top, top, MTOP[r0:r1])
                nc.vector.tensor_scalar_mul(bot, bot, MBOT[r0:r1])

            def stream_v(dst, wv_t, src, e):
                """dst[:, rng] = (wv_t.T @ src)[:, rng] via psA, DVE copy evac."""
                for lo, hi in chunks(exr(e), NT):
                    ps = psA.tile([128, NT], F32, tag="psA")
                    for l2, h2 in chunks((lo, hi), 512):
                        nc.tensor.matmul(
                            ps[:, l2 - lo : h2 - lo], wv_t[:], src[:, l2:h2],
                            start=True, stop=True,
                        )
                    nc.vector.tensor_copy(dst[:, lo:hi], ps[:, 0 : hi - lo])

            def conv_pe_gelu(dst, src, mats, e_out):
                """dst = gelu(dwconv(src)) over extent e_out, conv on PE.
                Uses double-buffered psA so gelu evac overlaps next chunk."""
                for lo, hi in chunks(exr(e_out), NT):
                    ps = psA.tile([128, NT], F32, tag="psA")
                    for l2, h2 in chunks((lo, hi), 512):
                        for t, (dy, dx) in enumerate(TAPS):
                            d = dy * RS + dx
                            nc.tensor.matmul(
                                ps[:, l2 - lo : h2 - lo],
                                mats[:, t * 128 : (t + 1) * 128],
                                src[:, l2 + d : h2 + d],
                                start=(t == 0), stop=(t == 8),
                            )
                    nc.scalar.activation(dst[:, lo:hi], ps[:, 0 : hi - lo], AF.Gelu)

            def conv_dve_gelu(dst, src, coef, e_out):
                lo, hi = exr(e_out)
                dy0, dx0 = TAPS[0]
                d0 = dy0 * RS + dx0
                nc.vector.tensor_scalar_mul(
                    dst[:, lo:hi], src[:, lo + d0 : hi + d0], coef[:, 0:1]
                )
                for t in range(1, 9):
                    dy, dx = TAPS[t]
                    d = dy * RS + dx
                    nc.vector.scalar_tensor_tensor(
                        dst[:, lo:hi], src[:, lo + d : hi + d], coef[:, t : t + 1],
                        dst[:, lo:hi], op0=ALU.mult, op1=ALU.add,
                    )
                nc.scalar.activation(dst[:, lo:hi], dst[:, lo:hi], AF.Gelu)

            def conv_dve_acc(dst, src, coef, e_out):
                for lo, hi in chunks(exr(e_out), NT):
                    scr_ = rp.tile([128, NT], BF16, tag="cscr", bufs=2)
                    n = hi - lo
                    dy0, dx0 = TAPS[0]
                    d0 = dy0 * RS + dx0
                    nc.vector.tensor_scalar_mul(
                        scr_[:, 0:n], src[:, lo + d0 : hi + d0], coef[:, 0:1]
                    )
                    for t in range(1, 9):
                        dy, dx = TAPS[t]
                        d = dy * RS + dx
                        nc.vector.scalar_tensor_tensor(
                            scr_[:, 0:n], src[:, lo + d : hi + d],
                            coef[:, t : t + 1], scr_[:, 0:n],
                            op0=ALU.mult, op1=ALU.add,
                        )
                    nc.vector.tensor_tensor(
                        dst[:, lo:hi], dst[:, lo:hi], scr_[:, 0:n], op=ALU.add
                    )

            def pair_tail(out_t, v_t, g_t, WoTp, pb_t, resid, pair, e_out, eng2):
                """out_t = out_c(v) [+ dw2(g) if pe] + pb + resid; then dve dw2."""
                mats = load_mdw(f"m_dw2_{pair}") if eng2 == "pe" else None
                for lo, hi in chunks(exr(e_out), NT):
                    ps = psA.tile([128, NT], F32, tag="psA")
                    for l2, h2 in chunks((lo, hi), 512):
                        nc.tensor.matmul(
                            ps[:, l2 - lo : h2 - lo], WoTp[:], v_t[:, l2:h2],
                            start=True, stop=(eng2 != "pe"),
                        )
                        if eng2 == "pe":
                            for t, (dy, dx) in enumerate(TAPS):
                                d = dy * RS + dx
                                nc.tensor.matmul(
                                    ps[:, l2 - lo : h2 - lo],
                                    mats[:, t * 128 : (t + 1) * 128],
                                    g_t[:, l2 + d : h2 + d],
                                    start=False, stop=(t == 8),
                                )
                    nc.vector.scalar_tensor_tensor(
                        out_t[:, lo:hi], ps[:, 0 : hi - lo], pb_t[:],
                        resid[:, lo:hi], op0=ALU.add, op1=ALU.add,
                    )
                if eng2 == "dve":
                    conv_dve_acc(out_t, g_t, WT[f"c_dw2_{pair}"], e_out)

            # =============== Gram1 + AR1 kickoff ===============
            arin = dp.tile([128, 2 * 128], F32, tag="arin")
            arout = dp.tile([128, 2 * 128], F32, tag="arout")
            arin_sb = sp.tile([128, 2 * 128], F32, tag="arin_sb")

            for gi, gname in enumerate(["gzx", "gzy"]):
                gps = psC.tile([128, 128], F32, tag="psC")
                for s in range(16):
                    t_ = rp.tile([128, 1024], BF16, tag="gch", bufs=2)
                    nc.sync.dma_start(t_[:], P[gname][s * 128 : (s + 1) * 128, :])
                    for k in range(8):
                        nc.tensor.matmul(
                            gps[:], t_[:, k * 128 : (k + 1) * 128],
                            t_[:, k * 128 : (k + 1) * 128],
                            start=(s == 0 and k == 0), stop=(s == 15 and k == 7),
                        )
                nc.vector.tensor_copy(arin_sb[:, gi * 128 : (gi + 1) * 128], gps[:])
            nc.sync.dma_start(arin[:], arin_sb[:])
            nc.gpsimd.collective_compute(
                "AllReduce", ALU.add, replica_groups=GROUPS,
                ins=[arin.opt()], outs=[arout.opt()],
            )

            # =============== load slabs; pre-attn conv work (hides AR1) =====
            xy = bp.tile([128, FA], BF16, tag="xy")
            nc.sync.dma_start(xy[:, RS : RS + R * RS], P["xy"][:])
            zz = bp.tile([128, FA], BF16, tag="s1")
            nc.sync.dma_start(zz[:, RS : RS + R * RS], P["zz"][:])

            v23 = bp.tile([128, FA], BF16, tag="s2")
            stream_v(v23, WT["wv23"], zz, 6)
            pad_zero(v23)
            g23 = bp.tile([128, FA], BF16, tag="s3")
            if ENG["dw1_23"] == "pe":
                conv_pe_gelu(g23, v23, load_mdw("m_dw1_23"), 5)
            else:
                conv_dve_gelu(g23, v23, WT["c_dw1_23"], 5)
            pad_zero(g23)
            mask_rows(g23, 5)

            # =============== attn L1 (needs AR1) ===============
            gmy = sp.tile([128, 2 * 128], F32, tag="gmy")
            nc.sync.dma_start(gmy[:], arout[:])

            def attn_module(m, G, qblk, kblk, WoTp, odd):
                """emit small-attn for module m from stack-Gram G [128, 128]."""
                r0 = 64 if odd else 0
                wq_s = WT["wqm"][0:62, m * 64 : m * 64 + 62]
                wk_s = WT["wkm"][0:62, m * 64 : m * 64 + 62]
                pw_s = WT["pwm"][0:62, m * 64 : m * 64 + 62]

                def blk(tag, rblk, cblk):
                    if rblk == 0:
                        return G[0:62, cblk * 64 : cblk * 64 + 62]
                    t_ = sp.tile([128, 64], F32, tag="gblk_" + tag)
                    nc.sync.dma_start(
                        t_[0:62, 0:62], G[64:126, cblk * 64 : cblk * 64 + 62]
                    )
                    return t_[0:62, 0:62]

                gqk = blk("qk", qblk, kblk)
                gqq = blk("qq", qblk, qblk)
                gkk = blk("kk", kblk, kblk)

                def mm62(lhs, rhs, tag):
                    pp = psC.tile([128, 128], F32, tag="psC")
                    nc.tensor.matmul(pp[0:62, 0:62], lhs, rhs, start=True, stop=True)
                    ss = sp.tile([128, 64], F32, tag="t_" + tag, name="t_" + tag)
                    nc.vector.tensor_copy(ss[0:62, 0:62], pp[0:62, 0:62])
                    return ss

                T1 = mm62(gqk, wq_s, "T1")
                T2 = mm62(gqq, wq_s, "T2")
                T3 = mm62(gkk, wk_s, "T3")
                SKQ = mm62(wk_s, T1[0:62, 0:62], "SKQ")
                u2 = sp.tile([128, 64], F32, tag="u2")
                nc.vector.tensor_tensor(
                    u2[0:62, 0:62], wq_s, T2[0:62, 0:62], op=ALU.mult
                )
                u3 = sp.tile([128, 64], F32, tag="u3")
                nc.vector.tensor_tensor(
                    u3[0:62, 0:62], wk_s, T3[0:62, 0:62], op=ALU.mult
                )
                pq = psC.tile([128, 128], F32, tag="psC")
                nc.tensor.matmul(
                    pq[0:1, 0:62], WT["ones62"][0:62, 0:1], u2[0:62, 0:62],
                    start=True, stop=True,
                )
                pk = psC.tile([128, 128], F32, tag="psC")
                nc.tensor.matmul(
                    pk[0:62, 0:1], u3[0:62, 0:62], WT["ones62"][0:62, 0:1],
                    start=True, stop=True,
                )
                ik = sp.tile([128, 2], F32, tag="ik")
                nc.scalar.activation(ik[0:62, 0:1], pk[0:62, 0:1], AF.Sqrt)
                nc.vector.tensor_scalar_max(ik[0:62, 0:1], ik[0:62, 0:1], 1e-12)
                nc.vector.reciprocal(ik[0:62, 1:2], ik[0:62, 0:1])
                scd = sp.tile([128, 1], F32, tag="scd")
                nc.vector.tensor_tensor(
                    scd[0:62, 0:1], ik[0:62, 1:2], WT["rsc"][0:62, m : m + 1],
                    op=ALU.mult,
                )
                iq = sp.tile([1, 128], F32, tag="iq")
                nc.scalar.activation(iq[0:1, 0:62], pq[0:1, 0:62], AF.Sqrt)
                nc.vector.tensor_scalar_max(iq[0:1, 0:62], iq[0:1, 0:62], 1e-12)
                nc.vector.reciprocal(iq[0:1, 64:126], iq[0:1, 0:62])
                iqb = sp.tile([128, 64], F32, tag="iqb")
                nc.gpsimd.partition_broadcast(iqb[0:62, 0:62], iq[0:1, 64:126])
                L = sp.tile([128, 64], F32, tag="L")
                nc.vector.tensor_scalar_mul(L[0:62, 0:62], SKQ[0:62, 0:62], scd[0:62, 0:1])
                nc.vector.tensor_tensor(
                    L[0:62, 0:62], L[0:62, 0:62], iqb[0:62, 0:62], op=ALU.mult
                )
                A = sp.tile([128, 64], F32, tag="A")
                nc.vector.memset(A[:], 0.0)
                nc.vector.tensor_tensor(
                    L[0:62, 0:62], L[0:62, 0:62], WT["hmA"][0:62, 0:62], op=ALU.mult
                )
                nc.vector.tensor_tensor(
                    L[0:62, 0:62], L[0:62, 0:62], WT["hmB"][0:62, 0:62], op=ALU.add
                )
                mx = sp.tile([128, 2], F32, tag="mx")
                nc.vector.tensor_reduce(
                    mx[0:62, 0:1], L[0:62, 0:62], op=ALU.max, axis=mybir.AxisListType.X
                )
                nc.vector.tensor_scalar_mul(mx[0:62, 1:2], mx[0:62, 0:1], -1.0)
                nc.scalar.activation(
                    A[0:62, 0:62], L[0:62, 0:62], AF.Exp, bias=mx[0:62, 1:2]
                )
                sm = sp.tile([128, 2], F32, tag="sm")
                nc.vector.tensor_reduce(
                    sm[0:62, 0:1], A[0:62, 0:62], op=ALU.add, axis=mybir.AxisListType.X
                )
                nc.vector.reciprocal(sm[0:62, 1:2], sm[0:62, 0:1])
                nc.vector.tensor_scalar_mul(A[0:62, 0:62], A[0:62, 0:62], sm[0:62, 1:2])
                wps = psC.tile([128, 128], F32, tag="psC")
                if odd:
                    nc.tensor.matmul(
                        wps[64:126, 0:62], A[0:62, 0:62], pw_s,
                        start=True, stop=True, tile_position=(0, 64),
                    )
                    nc.vector.tensor_copy(WoTp[64:126, 64:126], wps[64:126, 0:62])
                else:
                    nc.tensor.matmul(
                        wps[0:62, 0:62], A[0:62, 0:62], pw_s, start=True, stop=True
                    )
                    nc.vector.tensor_copy(WoTp[0:62, 0:62], wps[0:62, 0:62])

            WoT = {}
            for p in PAIRS:
                WoT[p] = sp.tile([128, 128], BF16, tag=f"WoT{p}", name=f"WoT{p}")
                nc.vector.memset(WoT[p][:], 0.0)
            # L1 attn: M0 (gzx stack: q=x blk1, k=x blk1), M1 (gzy: 1,1),
            # M2 (gzx: q=x 1, k=z 0), M3 (gzy: 1, 0)
            attn_module(0, gmy[:, 0:128], 1, 1, WoT["01"], odd=False)
            attn_module(1, gmy[:, 128:256], 1, 1, WoT["01"], odd=True)
            attn_module(2, gmy[:, 0:128], 1, 0, WoT["23"], odd=False)
            attn_module(3, gmy[:, 128:256], 1, 0, WoT["23"], odd=True)

            # =============== tails P23, P01 ===============
            fafb = bp.tile([128, FA], BF16, tag="s1")  # reuses zz slot
            pair_tail(fafb, v23, g23, WoT["23"], WT["pb23"], xy, "23", 4,
                      ENG["dw2_23"])
            pad_zero(fafb)
            mask_rows(fafb, 4)
            fafb_d = dp.tile([128, R * RS], BF16, tag="fafb_d")
            nc.sync.dma_start(fafb_d[:], fafb[:, RS : RS + R * RS])

            v01 = bp.tile([128, FA], BF16, tag="s2")
            stream_v(v01, WT["wv01"], xy, 4)
            pad_zero(v01)
            g01 = bp.tile([128, FA], BF16, tag="s3")
            if ENG["dw1_01"] == "pe":
                conv_pe_gelu(g01, v01, load_mdw("m_dw1_01"), 3)
            else:
                conv_dve_gelu(g01, v01, WT["c_dw1_01"], 3)
            pad_zero(g01)
            mask_rows(g01, 3)
            xaYb = bp.tile([128, FA], BF16, tag="s1")  # fafb spilled; reuse
            pair_tail(xaYb, v01, g01, WoT["01"], WT["pb01"], xy, "01", 2,
                      ENG["dw2_01"])
            pad_zero(xaYb)
            mask_rows(xaYb, 2)

            # =============== L2 Grams (PE transpose) + AR2 ===============
            fafb2 = bp.tile([128, FA], BF16, tag="s2")  # v01 slot dead
            nc.sync.dma_start(fafb2[:, RS : RS + R * RS], fafb_d[:])
            v45 = bp.tile([128, FA], BF16, tag="s3")  # g01 slot dead
            stream_v(v45, WT["wv45"], fafb2, 4)
            pad_zero(v45)

            arin2_sb = sp.tile([128, 2 * 128], F32, tag="arin_sb")
            base_off = (HALO + ROFF) * RS
            nck = OWN * RS // 128  # 129 chunks (incl pad cols = zeros)
            groups4 = [list(range(g, min(g + 4, nck))) for g in range(0, nck, 4)]
            for gi, r0 in enumerate([0, 64]):  # chain A, chain B
                gps = psC.tile([128, 128], F32, tag="psC")
                idn = WT["id64"][r0 : r0 + 64, 0:64]
                prev = None  # software pipeline: gram MMs lag transposes by 1 grp
                first = True
                for gidx, grp in enumerate(groups4):
                    # gram MMs of the previous group first: they fill the PE
                    # gap while this group's tp buffer waits on prev evac
                    if prev is not None:
                        pt, pn = prev
                        for k in range(pn):
                            nc.tensor.matmul(
                                gps[:], pt[:, k * 128 : (k + 1) * 128],
                                pt[:, k * 128 : (k + 1) * 128],
                                start=first, stop=False,
                            )
                            first = False
                    tp = psD.tile([128, 512], BF16, tag="tp", bufs=1)
                    for j, ck in enumerate(grp):
                        off = base_off + ck * 128
                        nc.tensor.transpose(
                            tp[:, j * 128 : j * 128 + 64],
                            xaYb[r0 : r0 + 64, off : off + 128], idn,
                        )
                        nc.tensor.transpose(
                            tp[:, j * 128 + 64 : j * 128 + 128],
                            fafb2[r0 : r0 + 64, off : off + 128], idn,
                        )
                    t_ = rp.tile([128, 512], BF16, tag="gch2", bufs=2)
                    n4 = len(grp) * 128
                    if gidx % 2 == 0:
                        nc.vector.tensor_copy(t_[:, 0:n4], tp[:, 0:n4])
                    else:
                        nc.scalar.copy(t_[:, 0:n4], tp[:, 0:n4])
                    prev = (t_, len(grp))
                pt, pn = prev
                for k in range(pn):
                    nc.tensor.matmul(
                        gps[:], pt[:, k * 128 : (k + 1) * 128],
                        pt[:, k * 128 : (k + 1) * 128],
                        start=False, stop=(k == pn - 1),
                    )
                nc.vector.tensor_copy(arin2_sb[:, gi * 128 : (gi + 1) * 128], gps[:])
            arin2 = dp.tile([128, 2 * 128], F32, tag="arin2")
            arout2 = dp.tile([128, 2 * 128], F32, tag="arout2")
            nc.sync.dma_start(arin2[:], arin2_sb[:])
            nc.gpsimd.collective_compute(
                "AllReduce", ALU.add, replica_groups=GROUPS,
                ins=[arin2.opt()], outs=[arout2.opt()],
            )

            # =============== P45 conv (hides AR2) ===============
            g45 = bp.tile([128, FA], BF16, tag="xy")  # xy slab dead after tails
            if ENG["dw1_45"] == "pe":
                conv_pe_gelu(g45, v45, load_mdw("m_dw1_45"), 3)
            else:
                conv_dve_gelu(g45, v45, WT["c_dw1_45"], 3)
            pad_zero(g45)
            mask_rows(g45, 3)

            gmy2 = sp.tile([128, 2 * 128], F32, tag="gmy")
            nc.sync.dma_start(gmy2[:], arout2[:])
            # stacks are [xa|fa]: q=xa blk0, k=fa blk1
            attn_module(4, gmy2[:, 0:128], 0, 1, WoT["45"], odd=False)
            attn_module(5, gmy2[:, 128:256], 0, 1, WoT["45"], odd=True)

            t12 = bp.tile([128, FA], BF16, tag="s2")  # fafb2 dead after v45/gram
            pair_tail(t12, v45, g45, WoT["45"], WT["pb45"], xaYb, "45", 2,
                      ENG["dw2_45"])
            pad_zero(t12)
            mask_rows(t12, 2)

            # =============== LN stats (chunk-interleaved) ===============
            xaYb_d = dp.tile([128, R * RS], BF16, tag="xaYb_d")
            nc.sync.dma_start(xaYb_d[:], xaYb[:, RS : RS + R * RS])

            lo2, hi2 = exr(2)
            lo1, hi1 = exr(1)
            L2p = 64 * 275  # 17600 staging per stat row
            stat_all_d = dp.tile([2, 2 * L2p], BF16, tag="stat_all_d")
            row_d = dp.tile([1, 2 * L2p], BF16, tag="row_d")

            for lo, hi in chunks((lo2, hi2), NT):
                n = hi - lo
                sqc = rp.tile([128, NT], BF16, tag="sqc", bufs=2)
                nc.scalar.activation(sqc[:, 0:n], t12[:, lo:hi], AF.Square)
                ps = psB.tile([128, NT], F32, tag="psB")
                ps2 = psA.tile([128, NT], F32, tag="psA")
                for l2, h2 in chunks((lo, hi), 512):
                    nc.tensor.matmul(
                        ps[0:2, l2 - lo : h2 - lo], WT["lnones"][:], t12[:, l2:h2],
                        start=True, stop=True,
                    )
                    nc.tensor.matmul(
                        ps2[0:2, l2 - lo : h2 - lo], WT["lnones"][:],
                        sqc[:, l2 - lo : h2 - lo],
                        start=True, stop=True,
                    )
                stch = rp.tile([2, 2 * NT], BF16, tag="stch", bufs=1)
                nc.scalar.copy(stch[:, 0:n], ps[0:2, 0:n])
                nc.scalar.copy(stch[:, NT : NT + n], ps2[0:2, 0:n])
                nc.sync.dma_start(
                    stat_all_d[:, lo - lo2 : hi - lo2], stch[:, 0:n]
                )
                nc.sync.dma_start(
                    stat_all_d[:, L2LEN + lo - lo2 : L2LEN + hi - lo2],
                    stch[:, NT : NT + n],
                )

            for ci, cn in ((0, "a"), (1, "b")):
                r0 = 0 if ci == 0 else 64
                mu64 = sp.tile([64, 275], BF16, tag="mu64")
                ms64 = sp.tile([64, 275], BF16, tag="ms64")
                nc.sync.dma_start(
                    mu64[:],
                    stat_all_d[ci : ci + 1, 0:L2p].rearrange(
                        "o (p f) -> (o p) f", p=64
                    ),
                )
                nc.sync.dma_start(
                    ms64[:],
                    stat_all_d[ci : ci + 1, L2LEN : L2LEN + L2p].rearrange(
                        "o (p f) -> (o p) f", p=64
                    ),
                )
                var = sp.tile([64, 275], F32, tag="var64")
                nc.vector.tensor_tensor(var[:], mu64[:], mu64[:], op=ALU.mult)
                nc.vector.tensor_tensor(var[:], ms64[:], var[:], op=ALU.subtract)
                sd = sp.tile([64, 275], F32, tag="sd64")
                nc.vector.tensor_scalar_add(var[:], var[:], 1e-5)
                nc.scalar.activation(sd[:], var[:], AF.Sqrt)
                inv = sp.tile([64, 275], F32, tag="inv64")
                nc.vector.reciprocal(inv[:], sd[:])
                invb = sp.tile([64, 275], BF16, tag="invb64")
                nc.vector.tensor_copy(invb[:], inv[:])
                musb = sp.tile([64, 275], BF16, tag="musb64")
                nc.vector.tensor_tensor(var[:], mu64[:], inv[:], op=ALU.mult)
                nc.vector.tensor_copy(musb[:], var[:])
                nc.sync.dma_start(
                    row_d[0:1, 0:L2p].rearrange("o (p f) -> (o p) f", p=64), invb[:]
                )
                nc.sync.dma_start(
                    row_d[0:1, L2p : 2 * L2p].rearrange("o (p f) -> (o p) f", p=64),
                    musb[:],
                )
                # broadcast s-hat, build ts
                sB = bp.tile([128, FA], BF16, tag="xy")  # g45/th1 slot
                nc.sync.dma_start(
                    sB[:, 0:L2LEN], row_d[0:1, 0:L2LEN].partition_broadcast(128)
                )
                ts = bp.tile([128, FA], BF16, tag="s1")
                nc.vector.tensor_tensor(
                    ts[r0 : r0 + 62, lo2:hi2], t12[r0 : r0 + 62, lo2:hi2],
                    sB[r0 : r0 + 62, 0:L2LEN], op=ALU.mult,
                )
                nc.sync.dma_start(
                    ts[r0 + 62 : r0 + 63, lo2:hi2], row_d[0:1, L2p : L2p + L2LEN]
                )
                th = {}
                for h, hn in ((0, "0"), (1, "1")):
                    tht = bp.tile([128, FA], BF16, tag=("s3" if h == 0 else "xy"))
                    th[h] = tht
                    w1t = WT[f"w1aug_{cn}{hn}"]
                    c1t = WT[f"c1b_{cn}{hn}"]
                    for lo, hi in chunks((lo2, hi2), NT):
                        ps = psA.tile([128, NT], F32, tag="psA")
                        for l2, h2 in chunks((lo, hi), 512):
                            nc.tensor.matmul(
                                ps[0:124, l2 - lo : h2 - lo], w1t[:], ts[:, l2:h2],
                                start=True, stop=True,
                            )
                        nc.scalar.activation(
                            tht[0:124, lo:hi], ps[0:124, 0 : hi - lo], AF.Gelu,
                            bias=c1t[0:124, :],
                        )
                    pad_zero(tht)
                    mask_rows(tht, 2, (0, 124))

                def ff_conv_chunk(h, hn, lo, hi, dst_ap):
                    """dst_ap[0:124, 0:hi-lo] = gelu(dwconv(th[h]))[lo:hi]."""
                    eng = ENG[f"ff_{cn}{hn}"]
                    if eng == "pe":
                        cps = psA.tile([128, NT], F32, tag="psA")
                        mats = load_mdw(f"m_ffdw_{cn}{hn}")
                        for l2, h2 in chunks((lo, hi), 512):
                            for t, (dy, dx) in enumerate(TAPS):
                                d = dy * RS + dx
                                nc.tensor.matmul(
                                    cps[:, l2 - lo : h2 - lo],
                                    mats[:, t * 128 : (t + 1) * 128],
                                    th[h][:, l2 + d : h2 + d],
                                    start=(t == 0), stop=(t == 8),
                                )
                        nc.scalar.activation(
                            dst_ap[0:124, 0 : hi - lo], cps[0:124, 0 : hi - lo],
                            AF.Gelu,
                        )
                    else:
                        coef = WT[f"c_ffdw_{cn}{hn}"]
                        dy0, dx0 = TAPS[0]
                        d0 = dy0 * RS + dx0
                        nc.vector.tensor_scalar_mul(
                            dst_ap[0:124, 0 : hi - lo],
                            th[h][0:124, lo + d0 : hi + d0], coef[0:124, 0:1],
                        )
                        for t in range(1, 9):
                            dy, dx = TAPS[t]
                            d = dy * RS + dx
                            nc.vector.scalar_tensor_tensor(
                                dst_ap[0:124, 0 : hi - lo],
                                th[h][0:124, lo + d : hi + d],
                                coef[0:124, t : t + 1],
                                dst_ap[0:124, 0 : hi - lo],
                                op0=ALU.mult, op1=ALU.add,
                            )
                        nc.scalar.activation(
                            dst_ap[0:124, 0 : hi - lo],
                            dst_ap[0:124, 0 : hi - lo], AF.Gelu,
                        )

                # FFa: all h0 conv chunks into freed ts slot (ts dead after w1)
                ghs0 = bp.tile([128, FA], BF16, tag="s1")
                for lo, hi in chunks((lo1, hi1), NT):
                    ff_conv_chunk(0, "0", lo, hi, ghs0[:, lo : lo + NT])
                # FFb: h1 conv (psA ring) + both w2 matmuls (psB) + STT add
                for lo, hi in chunks((lo1, hi1), NT):
                    ghc = rp.tile([128, NT], BF16, tag="ghc", bufs=2)
                    ff_conv_chunk(1, "1", lo, hi, ghc[:, 0:NT])
                    wps = psB.tile([128, NT], F32, tag="psB")
                    for l2, h2 in chunks((lo, hi), 512):
                        nc.tensor.matmul(
                            wps[r0 : r0 + 62, l2 - lo : h2 - lo],
                            WT[f"w2h_{cn}0"][0:124, 0:62],
                            ghs0[0:124, l2:h2],
                            start=True, stop=False,
                            tile_position=(0, 64) if ci == 1 else None,
                        )
                        nc.tensor.matmul(
                            wps[r0 : r0 + 62, l2 - lo : h2 - lo],
                            WT[f"w2h_{cn}1"][0:124, 0:62],
                            ghc[0:124, l2 - lo : h2 - lo],
                            start=False, stop=True,
                            tile_position=(0, 64) if ci == 1 else None,
                        )
                    nc.vector.scalar_tensor_tensor(
                        t12[r0 : r0 + 62, lo:hi], wps[r0 : r0 + 62, 0 : hi - lo],
                        1.0, t12[r0 : r0 + 62, lo:hi], op0=ALU.mult, op1=ALU.add,
                    )

            # =============== fusions + final ===============
            xaYb2 = bp.tile([128, FA], BF16, tag="s3")  # th0 slot dead
            nc.sync.dma_start(xaYb2[:, RS : RS + R * RS], xaYb_d[:])
            fab = bp.tile([128, FA], BF16, tag="xy")  # th1 slot dead
            for lo, hi in chunks((lo1, hi1), NT):
                ps = psA.tile([128, NT], F32, tag="psA")
                for l2, h2 in chunks((lo, hi), 512):
                    nc.tensor.matmul(
                        ps[:, l2 - lo : h2 - lo], WT["fuT"][:], t12[:, l2:h2],
                        start=True, stop=False,
                    )
                    nc.tensor.matmul(
                        ps[:, l2 - lo : h2 - lo], WT["fuX"][:], xaYb2[:, l2:h2],
                        start=False, stop=True,
                    )
                nc.scalar.add(fab[:, lo:hi], ps[:, 0 : hi - lo], WT["fucb"][:])
            pad_zero(fab)
            mask_rows(fab, 1)
            fin = bp.tile([128, FA], BF16, tag="s1")
            for lo, hi in chunks((lo1, hi1), NT):
                ps = psA.tile([128, NT], F32, tag="psA")
                for l2, h2 in chunks((lo, hi), 512):
                    nc.tensor.matmul(
                        ps[0:64, l2 - lo : h2 - lo], WT["outw"][:], fab[:, l2:h2],
                        start=True, stop=True,
                    )
                nc.scalar.add(fin[0:62, lo:hi], ps[0:62, 0 : hi - lo], WT["outb"][0:62, :])
            nc.sync.dma_start(fin[64:126, lo1:hi1], P["zc"][0:62, :])
            pad_zero(fin)
            mask_rows(fin, 1, (0, 62))
            out_stage = dp.tile([62, OWN * RS], F32, tag="out_stage")
            lo0, hi0 = exr(0)
            for lo, hi in chunks((lo0, hi0), NT):
                ps = psA.tile([128, NT], F32, tag="psA")
                for l2, h2 in chunks((lo, hi), 512):
                    for t in range(9):
                        dy, dx = TAPS[t]
                        d = dy * RS + dx
                        nc.tensor.matmul(
                            ps[0:64, l2 - lo : h2 - lo],
                            WT["finw"][:, t * 64 : (t + 1) * 64],
                            fin[:, l2 + d : h2 + d],
                            start=(t == 0), stop=(t == 8),
                        )
                for l3, h3 in chunks((lo, hi), 512):
                    och = rp.tile([62, 512], F32, tag="och", bufs=2)
                    nc.scalar.add(och[:, 0 : h3 - l3], ps[0:62, l3 - lo : h3 - lo], WT["finb"][0:62, :])
                    nc.sync.dma_start(
                        out_stage[:, l3 - lo0 : h3 - lo0], och[:, 0 : h3 - l3]
                    )
            nc.sync.dma_start(
                out_p[:].rearrange("c (r w) -> c r w", w=W),
                out_stage[:].rearrange("c (r s) -> c r s", s=RS)[:, :, 0:W],
            )

    nc.finalize()
    return nc


_NC_CACHE = {}


def _run(inputs, trace=False):
    if "nc" not in _NC_CACHE:
        _NC_CACHE["nc"] = build_nc()
    nc = _NC_CACHE["nc"]
    names = {
        a.name.removesuffix("_set")
        for a in nc.m.functions[0].allocations
        if getattr(a, "kind", None) == "ExternalInput"
    }
    in_maps = prep_host_inputs(inputs)
    in_maps = [{k: v for k, v in m.items() if k in names} for m in in_maps]
    res = run_bass_kernel_spmd(
        nc, in_maps, core_ids=list(range(8)), trace=trace
    )
    return assemble_output(res.results), res


def kernel(**inputs):
    out, _ = _run(inputs, trace=False)
    return out
